# revision 1
# baseline (speedup 1.0000x reference)
"""AttentionHeadVDP Trainium2 kernel — 8-core SPMD.

Sharding: data-parallel over batch (2) x tensor-parallel over heads (4 groups
of 4 heads) = 8 cores. Each core computes its batch's [S, 256] slice of the
output mean/variance; host gathers, transposes back, and concatenates.

Per-core device algorithm (all layouts feature-major/"transposed" so every
matmul contraction lands on the partition dim with zero on-device transposes):
  host-prepped inputs: xt = x[b].T, vxt = var_x[b].T, x2t = (x[b].T)^2 (bf16);
    per proj P: wm = Wp.T, wa = (Wp_var + Wp^2).T, wv = Wp_var.T (bf16 slices);
    xres = x[b][:, J].T (f32)
  projections (PE, bf16): QT = wm_q.T @ xt [256, S]; VQT = wa_q.T@vxt +
    wv_q.T@x2t; same for K; V / VV natural [S, 256]
  per head h, scores transposed: aT[s2,s1] = KT_h.T @ QT_h (1/sqrt(D) folded
    into the exp scale); vaT = (VKT+KT^2)_h.T @ VQT_h + VKT_h.T @ QT^2_h
  e = exp(aT/32), e2 = exp(2 aT/32) (bf16); o^T_un = [V_h | 1s].T @ e, whose
    row 64 = softmax denominators; r = 1/sums; R bcast via K=1 fp32r matmul
  vp_un = Square((e*R - 1)/32) * e2 * vaT  (the 1/32^2 folds the score-var
    1/1024); vo^T_un = (VV+V^2)_h.T @ vp_un + VV_h.T @ e2
  unnormalized-softmax powers fold into fp32 output scales:
    O = o^T_un * r + xres ;  VO = (vo^T_un * r) * r

Structure: two sequential pool epochs (scheduler-safe zone reuse):
  epoch 1: input loads + all projections (proj PSUM pool, 6 banks)
  epoch 2: per head h: A(h) scores/exp/PV-mu -> gst(h) g/sq1/t -> B(h)
    var-scores/vp/PV-var.  PSUM: tag sc [128,1024] x2 (mu/va/R1) + tag acc
    [65,1024] x2 (omu/ova) = 8 banks.
"""

import os
import sys
from contextlib import ExitStack

for _p in ("/opt/trn_rl_repo", "/root/.axon_site/_ro/trn_rl_repo"):
    if os.path.isdir(_p) and _p not in sys.path:
        sys.path.insert(0, _p)

import numpy as np
import ml_dtypes

import concourse.bass as bass
import concourse.mybir as mybir
import concourse.tile as tile
from concourse import bacc
from concourse.bass_utils import run_bass_kernel_spmd

BF16 = mybir.dt.bfloat16
F32 = mybir.dt.float32
F32R = mybir.dt.float32r
AF = mybir.ActivationFunctionType
OP = mybir.AluOpType

B, S, D = 2, 1024, 1024
NHEAD_TOTAL, DH = 16, 64
HPC = 4           # heads per core
J = HPC * DH      # 256 output cols per core
NK = S // 128     # 8 partition tiles

_CACHE = {}


def _build_program():
    nc = bacc.Bacc("TRN2", target_bir_lowering=False, debug=False, num_devices=8)

    din = {}
    for nm in ("xt", "vxt"):
        din[nm] = nc.dram_tensor(nm, [D, S], BF16, kind="ExternalInput")
    for p in "qkv":
        for sfx in ("m", "a", "v"):
            nm = f"w{p}_{sfx}"
            din[nm] = nc.dram_tensor(nm, [D, J], BF16, kind="ExternalInput")
    din["xres"] = nc.dram_tensor("xres", [J, S], F32, kind="ExternalInput")
    din["ones_in"] = nc.dram_tensor("ones_in", [1, 128], F32R, kind="ExternalInput")
    o_mu_d = nc.dram_tensor("o_mu", [J, S], F32, kind="ExternalOutput")
    o_va_d = nc.dram_tensor("o_va", [J, S], F32, kind="ExternalOutput")

    with ExitStack() as ctx:
        tc = ctx.enter_context(tile.TileContext(nc))

        # ---- persistent SBUF tensors (whole kernel) ----
        pp = ctx.enter_context(tc.tile_pool(name="persist", bufs=1))
        QT = [pp.tile([128, S], BF16, tag=f"QT{j}", name=f"QT{j}") for j in range(2)]
        VQT = [pp.tile([128, S], BF16, tag=f"VQT{j}", name=f"VQT{j}") for j in range(2)]
        KT = [pp.tile([128, S], BF16, tag=f"KT{j}", name=f"KT{j}") for j in range(2)]
        VKT = [pp.tile([128, S], BF16, tag=f"VKT{j}", name=f"VKT{j}") for j in range(2)]
        Q2T = [pp.tile([128, S], BF16, tag=f"Q2T{j}", name=f"Q2T{j}") for j in range(2)]
        KVK = [pp.tile([128, S], BF16, tag=f"KVK{j}", name=f"KVK{j}") for j in range(2)]
        V_sb = [pp.tile([128, 65 * HPC], BF16, tag=f"V{i}", name=f"V{i}") for i in range(NK)]
        VV_sb = [pp.tile([128, J], BF16, tag=f"VV{i}", name=f"VV{i}") for i in range(NK)]
        VVpV2 = [pp.tile([128, J], BF16, tag=f"VVp{i}", name=f"VVp{i}") for i in range(NK)]
        O_all = {}
        VO_all = {}
        xres_sb = {}
        ones_sb = pp.tile([1, 128], F32R, tag="ones", name="ones")
        nc.sync.dma_start(ones_sb[:, :], din["ones_in"][:, :])
        sqbias = pp.tile([128, 1], F32, tag="sqbias", name="sqbias")
        nc.gpsimd.memset(sqbias[:, :], -1.0 / 32.0)

        # ================= epoch 1: xt + mu-weights, QT/KT ==================
        xtp = ctx.enter_context(tc.tile_pool(name="xtp", bufs=1))
        xs = {}
        xs["xt"] = []
        for k in range(NK):
            t = xtp.tile([128, S], BF16, tag=f"xt{k}", name=f"xt{k}")
            nc.sync.dma_start(t[:, :], din["xt"][128 * k:128 * k + 128, :])
            xs["xt"].append(t)
        with tc.tile_pool(name="inpA", bufs=1) as inpA, \
             tc.tile_pool(name="projps", bufs=2, space="PSUM") as pps:
            ws = {}
            for nm in ("wq_m", "wk_m"):
                ws[nm] = []
                for k in range(NK):
                    t = inpA.tile([128, J], BF16, tag=f"{nm}{k}", name=f"{nm}{k}")
                    nc.sync.dma_start(t[:, :], din[nm][128 * k:128 * k + 128, :])
                    ws[nm].append(t)

            def fm_proj_1(dst, terms, eng, pool, tag, jt):
                # [256, S] projection for one jt half: one [128,1024] psum
                # whose two n-halves accumulate independently; lhsT shared.
                ngrp = len(terms) * NK
                ps = pool.tile([128, S], F32, tag=tag, name="fmps")
                i = 0
                for k in range(NK):
                    for wnm, xnm in terms:
                        for n in range(2):
                            nc.tensor.matmul(
                                ps[:, 512 * n:512 * n + 512],
                                ws[wnm][k][:, 128 * jt:128 * jt + 128],
                                xs[xnm][k][:, 512 * n:512 * n + 512],
                                start=(i == 0), stop=(i == ngrp - 1),
                            )
                        i += 1
                for n in range(2):
                    nsl = slice(512 * n, 512 * n + 512)
                    if eng == "act":
                        nc.scalar.copy(dst[jt][:, nsl], ps[:, nsl])
                    else:
                        nc.vector.tensor_copy(dst[jt][:, nsl], ps[:, nsl])

            fm_proj_1(QT, [("wq_m", "xt")], "act", pps, "pj", 0)
            fm_proj_1(KT, [("wk_m", "xt")], "act", pps, "pj", 0)
            fm_proj_1(QT, [("wq_m", "xt")], "act", pps, "pj", 1)
            fm_proj_1(KT, [("wk_m", "xt")], "act", pps, "pj", 1)

        # ================= epoch 2: attention + var projections =============
        # PSUM: scmu x1 [128,1024] (mu/R1) + scva x1 (va + var-proj accums)
        # + acc x2 [65,1024] (omu/ova) = 8 banks.
        psp = ctx.enter_context(tc.tile_pool(name="ps", bufs=1, space="PSUM"))
        inpB = ctx.enter_context(tc.tile_pool(name="inpB", bufs=1))
        est = ctx.enter_context(tc.tile_pool(name="est", bufs=16))
        ep = ctx.enter_context(tc.tile_pool(name="epool", bufs=9))
        chain = ctx.enter_context(tc.tile_pool(name="chain", bufs=2))
        rp = ctx.enter_context(tc.tile_pool(name="rpool", bufs=2))
        t64p = ctx.enter_context(tc.tile_pool(name="tmp64", bufs=1))
        p0t = ctx.enter_context(tc.tile_pool(name="ph0tmp", bufs=1))
        outp = ctx.enter_context(tc.tile_pool(name="outp", bufs=1))

        def open_jt(jt):
            O_all[jt] = outp.tile([128, S], F32, tag="O", name=f"O{jt}")
            VO_all[jt] = outp.tile([128, S], F32, tag="VO", name=f"VO{jt}")
            xres_sb[jt] = outp.tile([128, S], F32, tag="xres", name=f"xres{jt}")
            nc.sync.dma_start(xres_sb[jt][:, :], din["xres"][128 * jt:128 * jt + 128, :])
        open_jt(0)

        xs["vxt"] = []
        for k in range(NK):
            t = inpB.tile([128, S], BF16, tag=f"vxt{k}", name=f"vxt{k}")
            nc.sync.dma_start(t[:, :], din["vxt"][128 * k:128 * k + 128, :])
            xs["vxt"].append(t)
        xs["x2t"] = []
        for k in range(NK):
            t = inpB.tile([128, S], BF16, tag=f"x2t{k}", name=f"x2t{k}")
            nc.gpsimd.tensor_tensor(t[:, :], xs["xt"][k][:, :], xs["xt"][k][:, :], OP.mult)
            xs["x2t"].append(t)
        for nm in ("wv_m", "wq_a", "wq_v", "wk_a", "wk_v", "wv_a", "wv_v"):
            ws[nm] = []
            for k in range(NK):
                t = inpB.tile([128, J], BF16, tag=f"{nm}{k}", name=f"{nm}{k}")
                nc.sync.dma_start(t[:, :], din[nm][128 * k:128 * k + 128, :])
                ws[nm].append(t)

        def v_nat():
            # natural-layout V [S, 256]: st pairs, scva slots
            for stg in range(4):
                vps = [psp.tile([128, 512], F32, tag="scva", name="natv", bufs=2)
                       for _ in range(2)]
                for k in range(NK):
                    for st2 in range(2):
                        st = 2 * stg + st2
                        nc.tensor.matmul(vps[st2][:, 0:J],
                                         xs["xt"][k][:, 128 * st:128 * st + 128],
                                         ws["wv_m"][k][:, :],
                                         start=(k == 0), stop=(k == NK - 1))
                for st2 in range(2):
                    st = 2 * stg + st2
                    for h in range(HPC):
                        nc.vector.tensor_copy(V_sb[st][:, 65 * h:65 * h + 64],
                                              vps[st2][:, 64 * h:64 * h + 64])
                    vone = V_sb[st][:, :].rearrange("p (h c) -> p h c", h=HPC)[:, :, 64:65]
                    nc.gpsimd.memset(vone, 1.0)
                yield

        def var_projs():
            # generator: yields between chunks so emission can interleave
            def fm_proj_2(dst, terms, eng):
                # sub-generator: yields after each jt half
                ngrp = len(terms) * NK
                for jt in range(2):
                    ps = [psp.tile([128, 512], F32, tag="scva", name="fmps2", bufs=2)
                          for _ in range(2)]
                    i = 0
                    for k in range(NK):
                        for wnm, xnm in terms:
                            for n in range(2):
                                nc.tensor.matmul(
                                    ps[n][:, :],
                                    ws[wnm][k][:, 128 * jt:128 * jt + 128],
                                    xs[xnm][k][:, 512 * n:512 * n + 512],
                                    start=(i == 0), stop=(i == ngrp - 1),
                                )
                            i += 1
                    for n in range(2):
                        nsl = slice(512 * n, 512 * n + 512)
                        if eng == "act":
                            nc.scalar.copy(dst[jt][:, nsl], ps[n][:, :])
                        else:
                            nc.vector.tensor_copy(dst[jt][:, nsl], ps[n][:, :])
                    yield

            yield from fm_proj_2(VQT, [("wq_a", "vxt"), ("wq_v", "x2t")], "dve")
            yield from fm_proj_2(VKT, [("wk_a", "vxt"), ("wk_v", "x2t")], "dve")
            for jt in range(2):
                nc.vector.tensor_tensor(Q2T[jt][:, :], QT[jt][:, :], QT[jt][:, :], OP.mult)
                k2 = p0t.tile([128, S], BF16, tag="k2", name="k2")
                nc.vector.tensor_tensor(k2[:, :], KT[jt][:, :], KT[jt][:, :], OP.mult)
                nc.vector.tensor_tensor(KVK[jt][:, :], k2[:, :], VKT[jt][:, :], OP.add)
            yield
            for stg in range(4):
                pv2 = [psp.tile([128, 512], F32, tag="scva", name="natvv", bufs=2)
                       for _ in range(2)]
                for k in range(NK):
                    for wnm, xnm in (("wv_a", "vxt"), ("wv_v", "x2t")):
                        for st2 in range(2):
                            st = 2 * stg + st2
                            nc.tensor.matmul(pv2[st2][:, 0:J],
                                             xs[xnm][k][:, 128 * st:128 * st + 128],
                                             ws[wnm][k][:, :],
                                             start=(k == 0 and wnm == "wv_a"),
                                             stop=(k == NK - 1 and wnm == "wv_v"))
                for st2 in range(2):
                    st = 2 * stg + st2
                    nc.vector.tensor_copy(VV_sb[st][:, :], pv2[st2][:, 0:J])
                    v2 = p0t.tile([128, J], BF16, tag="v2", name="v2")
                    vdat = V_sb[st][:, :].rearrange("p (h c) -> p h c", h=HPC)[:, :, 0:64]
                    v2v = v2[:, :].rearrange("p (h c) -> p h c", h=HPC)
                    nc.vector.tensor_tensor(v2v, vdat, vdat, OP.mult)
                    nc.vector.tensor_tensor(VVpV2[st][:, :], v2[:, :], VV_sb[st][:, :], OP.add)
                yield

        e_tiles = {}
        e2_tiles = {}
        t_tiles = {}
        R1_tiles = {}
        Rb_tiles = {}
        omu_tiles = {}
        ova_tiles = {}

        def a_strip(h, i):
            jt, po = h // 2, 64 * (h % 2)
            psl = slice(po, po + 64)
            if i == 0:
                e_tiles[h] = []
                e2_tiles[h] = []
                omu_tiles[h] = psp.tile([65, S], F32, tag="acc", name="omu", bufs=2)
            omu_ps = omu_tiles[h]
            isl = slice(128 * i, 128 * i + 128)
            mu_ps = psp.tile([128, S], F32, tag="scmu", name="mu")
            for n in range(2):
                nsl = slice(512 * n, 512 * n + 512)
                nc.tensor.matmul(mu_ps[:, nsl], KT[jt][psl, isl],
                                 QT[jt][psl, nsl], start=True, stop=True)
            e_i = ep.tile([128, S], BF16, tag="e", name="e")
            nc.scalar.activation(e_i[:, :], mu_ps[:, :], AF.Exp, scale=1.0 / 32.0)
            e2_i = est.tile([128, S], BF16, tag="est", name="e2")
            nc.gpsimd.tensor_tensor(e2_i[:, :], e_i[:, :], e_i[:, :], OP.mult)
            for n in range(2):
                nsl = slice(512 * n, 512 * n + 512)
                nc.tensor.matmul(omu_ps[:, nsl], V_sb[i][:, 65 * h:65 * h + 65],
                                 e_i[:, nsl], start=(i == 0), stop=(i == NK - 1))
            e_tiles[h].append(e_i)
            e2_tiles[h].append(e2_i)

        def a_tail(h):
            jt, po = h // 2, 64 * (h % 2)
            psl = slice(po, po + 64)
            omu_ps = omu_tiles[h]
            r_sb = rp.tile([1, S], F32R, tag="r", name="r", bufs=1)
            with nc.allow_low_precision(reason="fp32r r for PE broadcast"):
                nc.vector.reciprocal(r_sb[:, :], omu_ps[64:65, :])
            R1_ps = psp.tile([128, S], F32, tag="scmu", name="R1ps")
            for n in range(2):
                nsl = slice(512 * n, 512 * n + 512)
                nc.tensor.matmul(R1_ps[:, nsl], ones_sb[:, :], r_sb[:, nsl],
                                 start=True, stop=True)
            R1_sb = rp.tile([128, S], F32, tag="R1", name="R1")
            Rb_sb = rp.tile([128, S], BF16, tag="Rb", name="Rb", bufs=1)
            nc.scalar.copy(R1_sb[:, :], R1_ps[:, :])
            nc.scalar.copy(Rb_sb[:, :], R1_ps[:, :])
            R1_tiles[h] = R1_sb
            Rb_tiles[h] = Rb_sb
            # O = o_un * r + xres  (R1 rows identical: any 64-row slice works)
            t64 = t64p.tile([128, S], F32, tag="t64", name="t64")
            nc.vector.tensor_tensor(t64[psl, :], omu_ps[0:64, :], R1_sb[psl, :], OP.mult)
            nc.gpsimd.tensor_tensor(O_all[jt][psl, :], t64[psl, :], xres_sb[jt][psl, :], OP.add)

        def gst_strip(h, i):
            # g = e*R ; sq1 = Square((g-1)/32) ; t = sq1 * e2
            if i == 0:
                t_tiles[h] = []
            Rb_sb = Rb_tiles[h]
            e_i = e_tiles[h][i]
            g_i = chain.tile([128, S], BF16, tag="g", name="g")
            nc.vector.tensor_tensor(g_i[:, :], e_i[:, :], Rb_sb[:, :], OP.mult)
            sq_i = chain.tile([128, S], BF16, tag="sq", name="sq")
            nc.scalar.activation(sq_i[:, :], g_i[:, :], AF.Square,
                                 scale=1.0 / 32.0, bias=sqbias[:, :])
            t_i = est.tile([128, S], BF16, tag="est", name="t")
            nc.vector.tensor_tensor(t_i[:, :], sq_i[:, :], e2_tiles[h][i][:, :], OP.mult)
            t_tiles[h].append(t_i)

        def b_strip(h, i):
            jt, po = h // 2, 64 * (h % 2)
            psl = slice(po, po + 64)
            if i == 0:
                ova_tiles[h] = psp.tile([65, S], F32, tag="acc", name="ova", bufs=2)
            ova_ps = ova_tiles[h]
            isl = slice(128 * i, 128 * i + 128)
            vp_i = chain.tile([128, S], BF16, tag="vp", name="vp")
            for n in range(2):
                nsl = slice(512 * n, 512 * n + 512)
                va_ps = psp.tile([128, 512], F32, tag="scva", name="va", bufs=2)
                nc.tensor.matmul(va_ps[:, :], KVK[jt][psl, isl],
                                 VQT[jt][psl, nsl], start=True, stop=False)
                nc.tensor.matmul(va_ps[:, :], VKT[jt][psl, isl],
                                 Q2T[jt][psl, nsl], start=False, stop=True)
                nc.vector.tensor_tensor(vp_i[:, nsl], t_tiles[h][i][:, nsl],
                                        va_ps[:, :], OP.mult)
                nc.tensor.matmul(ova_ps[0:64, nsl], VVpV2[i][:, 64 * h:64 * h + 64],
                                 vp_i[:, nsl], start=(i == 0), stop=False)
                nc.tensor.matmul(ova_ps[0:64, nsl], VV_sb[i][:, 64 * h:64 * h + 64],
                                 e2_tiles[h][i][:, nsl], start=False,
                                 stop=(i == NK - 1))

        def b_tail(h):
            jt, po = h // 2, 64 * (h % 2)
            psl = slice(po, po + 64)
            ova_ps = ova_tiles[h]
            R1_sb = R1_tiles[h]
            t64b = t64p.tile([128, S], F32, tag="t64", name="t64b")
            nc.vector.tensor_tensor(t64b[psl, :], ova_ps[0:64, :], R1_sb[psl, :], OP.mult)
            nc.gpsimd.tensor_tensor(VO_all[jt][psl, :], t64b[psl, :], R1_sb[psl, :], OP.mult)

        # software-pipelined schedule: head h's scores/exp stream while head
        # h-1's var chain + PV-var run; var-projections weave into head 0.
        vn = iter(v_nat())
        next(vn, None)
        vp_chunks = iter(var_projs())
        for i in range(NK):
            if i % 2 == 1 and i < 7:
                next(vn, None)
            a_strip(0, i)
            if i >= 1:
                next(vp_chunks, None)
        a_tail(0)
        for _ in vn:
            pass
        for _ in vp_chunks:
            pass
        for h in range(1, HPC):
            for i in range(NK):
                a_strip(h, i)
                gst_strip(h - 1, i)
                b_strip(h - 1, i)
            a_tail(h)
            b_tail(h - 1)
            if h == 1:
                open_jt(1)
            if h == 2:
                nc.sync.dma_start(o_mu_d[0:128, :], O_all[0][:, :])
                nc.sync.dma_start(o_va_d[0:128, :], VO_all[0][:, :])
        for i in range(NK):
            gst_strip(HPC - 1, i)
            b_strip(HPC - 1, i)
        b_tail(HPC - 1)

        nc.sync.dma_start(o_mu_d[128:256, :], O_all[1][:, :])
        nc.sync.dma_start(o_va_d[128:256, :], VO_all[1][:, :])

    nc.compile()
    return nc


def _get_program():
    if "nc" not in _CACHE:
        _CACHE["nc"] = _build_program()
    return _CACHE["nc"]


def _bf(a):
    return np.ascontiguousarray(a.astype(np.float32), dtype=np.float32).astype(ml_dtypes.bfloat16)


def _prep_core(core, x, var_x, wdict):
    b, g = core // 4, core % 4
    Jsl = slice(256 * g, 256 * g + 256)
    xb = np.asarray(x[b], np.float32)
    vxb = np.asarray(var_x[b], np.float32)
    m = {
        "xt": _bf(xb.T),
        "vxt": _bf(vxb.T),
        "xres": np.ascontiguousarray(xb[:, Jsl].T),
        "ones_in": np.ones((1, 128), np.float32),
    }
    for p in "qkv":
        W = np.asarray(wdict[f"w{p}_mu"][Jsl, :], np.float32)
        Wv = np.asarray(wdict[f"w{p}_var"][Jsl, :], np.float32)
        m[f"w{p}_m"] = _bf(W.T)
        m[f"w{p}_a"] = _bf((Wv + W * W).T)
        m[f"w{p}_v"] = _bf(Wv.T)
    return m


def kernel(x, var_x, wq_mu, wq_var, wk_mu, wk_var, wv_mu, wv_var, _trace=False):
    x = np.asarray(x, np.float32)
    var_x = np.asarray(var_x, np.float32)
    nc = _get_program()
    wdict = dict(wq_mu=wq_mu, wq_var=wq_var, wk_mu=wk_mu, wk_var=wk_var,
                 wv_mu=wv_mu, wv_var=wv_var)
    in_maps = [_prep_core(c, x, var_x, wdict) for c in range(8)]
    res = run_bass_kernel_spmd(nc, in_maps, core_ids=list(range(8)), trace=_trace)
    out_mu = np.empty((B, S, D), np.float32)
    out_va = np.empty((B, S, D), np.float32)
    for c in range(8):
        b, g = c // 4, c % 4
        Jsl = slice(256 * g, 256 * g + 256)
        out_mu[b, :, Jsl] = res.results[c]["o_mu"].T
        out_va[b, :, Jsl] = res.results[c]["o_va"].T
    if _trace:
        _CACHE["last_results"] = res
    return out_mu, out_va

